# revision 9
# baseline (speedup 1.0000x reference)
"""Batched Kalman filter for Trainium2 (Bass), 8-core data parallel.

The reference filter's P/K evolution is data- and batch-independent, so the
per-step gains can be computed on the host. When every per-step update matrix
is a scalar multiple of the identity (true for the shipped identity
parameters), the whole filter collapses to

    out[b] = W @ y[b]        W[t, s] = b_s * prod_{r=s+1..t} a_r   (lower-tri)

with a_t = 1 - k_t, b_t = k_t from the scalar gain recursion. On device this
is a single [64, 64] weight matmul applied per batch element: time-major
layout puts the contraction axis (s) on partitions, so each batch element's
[64, 64] block streams through the PE array with the weight stationary.

The device kernel is HBM-bound (read every input byte, write every output
byte), so the data is moved as fp16 and laid out on the host in the exact
SBUF image the kernel consumes: a [128, 65536] block per core whose row
q = h*64 + s holds time-step s of the batch elements with parity h, and
whose columns are (pair-index, feature). Every DMA is then a plain 2D slice
with 8 KB contiguous per partition — full-rate descriptors in both
directions — and the matmul runs at fp16 speed with fp32 PSUM accumulation.
The host repacks the fp16 result to [B, T, D] float32.
"""

import numpy as np

B = 16384
NCORES = 8
BS = B // NCORES          # 2048 batch rows per core

T = 64
D = 64

NPAIR = BS // 2           # 1024 batch pairs per core
NCOL = NPAIR * D          # 65536 columns in the packed per-core image

SLOT = 4096               # columns per slab (8 KB/partition fp16)
NSLAB = NCOL // SLOT      # 16
MM_N = 512                # matmul free size (one PSUM bank)
MM_PER_SLAB = SLOT // MM_N   # 8
XBUFS = 6                 # x-slab slots resident in SBUF
OBUFS = 6                 # out-slab slots resident in SBUF

_CACHE = {}


SECTOR = 1024             # matmul/copy unit: 2 PSUM banks of fp32
NSEC = NCOL // SECTOR     # 64
SEC_PER_SLAB = SLOT // SECTOR  # 4
MM_PER_SEC = SECTOR // MM_N    # 2
MM_PER_SLAB = SLOT // MM_N     # 8
PSUM_DEPTH = 4            # sectors in flight in PSUM (4 x 1024 f32 = 8 banks)

# --- static schedule tables (tuned against the CoreSim cost model) --------
# Load engine per slab: "sp" or "act" (both HWDGE channels).
LOAD_ENG = ["act" if s % 3 == 2 else "sp" for s in range(NSLAB)]
# Store engine per slab: "pool" (SWDGE) or "act".
STORE_ENG = ["act" if s % 3 == 1 else "pool" for s in range(NSLAB)]
# Copy engine per sector, pattern of 16 (A=scalar, D=vector, P=gpsimd).
_CP = "ADAD" "DAPD" "ADAP" "DAPD"
COPY_ENG = [{"A": "act", "D": "dve", "P": "pool"}[_CP[k % 16]]
            for k in range(NSEC)]
LOAD_PREFETCH = 4         # issue act loads this many slabs ahead


def build_nc():
    import concourse.bass as bass
    import concourse.mybir as mybir

    f16 = mybir.dt.float16
    f32 = mybir.dt.float32

    nc = bass.Bass()
    x = nc.declare_dram_parameter("x", [128, NCOL], f16, isOutput=False)
    w = nc.declare_dram_parameter("w", [128, 128], f16, isOutput=False)
    out = nc.declare_dram_parameter("out", [128, NCOL], f16, isOutput=True)

    # DMA-channel sem rings: each DMA class uses RING sems round-robin; the
    # j-th DMA waits ring[j%RING] >= 16*(j//RING) before issuing (proving the
    # ring slot's previous DMA landed — orders same-sem increments for the
    # race detector) and incs ring[j%RING] to 16*(j//RING+1) on completion.
    RING = 3
    # tokens: (sem_name, value) proving completion
    in_token = [None] * NSLAB       # load of slab s landed
    in_guard = [None] * NSLAB       # wait required before issuing load s
    store_token = [None] * NSLAB    # store of slab s completed
    store_guard = [None] * NSLAB
    copy_token = [None] * NSEC      # copy of sector k completed
    n_in = {"sp": 0, "act": 0}
    n_st = {"pool": 0, "act": 0}
    n_cp = {"act": 0, "dve": 0, "pool": 0}
    for s in range(NSLAB):
        e = LOAD_ENG[s]
        j = n_in[e]; n_in[e] += 1
        nm = f"{e}_in{j % RING}"
        in_token[s] = (nm, 16 * (j // RING + 1))
        in_guard[s] = (nm, 16 * (j // RING)) if j >= RING else None
        e = STORE_ENG[s]
        j = n_st[e]; n_st[e] += 1
        nm = f"{e}_out{j % RING}"
        store_token[s] = (nm, 16 * (j // RING + 1))
        store_guard[s] = (nm, 16 * (j // RING)) if j >= RING else None
    for k in range(NSEC):
        e = COPY_ENG[k]
        n_cp[e] += 1
        copy_token[k] = (e + "_cp", n_cp[e])

    ring_names = [f"{e}_{d}{i}" for e in ("sp", "act", "pool")
                  for d in ("in", "out") for i in range(RING)
                  if not (e == "sp" and d == "out") and not (e == "pool" and d == "in")]

    from contextlib import ExitStack

    with ExitStack() as stack:
        xt = stack.enter_context(nc.sbuf_tensor([128, XBUFS * SLOT], f16))
        ot = stack.enter_context(nc.sbuf_tensor([128, OBUFS * SLOT], f16))
        wt = stack.enter_context(nc.sbuf_tensor([128, 128], f16))
        pt = stack.enter_context(nc.psum_tensor([128, PSUM_DEPTH * SECTOR], f32))
        w_sem = stack.enter_context(nc.semaphore("w_sem"))
        pe_sem = stack.enter_context(nc.semaphore("pe_sem"))
        sems = {}
        for nm in ring_names + ["act_cp", "dve_cp", "pool_cp"]:
            sems[nm] = stack.enter_context(nc.semaphore(nm))
        block = stack.enter_context(nc.Block())

        def x_slot(s):
            s0 = (s % XBUFS) * SLOT
            return xt[:, s0:s0 + SLOT]

        def o_slot(s):
            s0 = (s % OBUFS) * SLOT
            return ot[:, s0:s0 + SLOT]

        def psum_sec(k):
            c0 = (k % PSUM_DEPTH) * SECTOR
            return pt[:, c0:c0 + SECTOR]

        def wait_token(eng, tok):
            eng.wait_ge(sems[tok[0]], tok[1])

        def emit_load(eng, s):
            """Issue the load for slab s on `eng` (with slot-recycle guards)."""
            if s >= XBUFS:
                # slot free once slab s-XBUFS's matmuls are done; the
                # in-token wait also proves the slot's previous load landed
                eng.wait_ge(pe_sem, MM_PER_SLAB * (s - XBUFS + 1))
                wait_token(eng, in_token[s - XBUFS])
            if in_guard[s] is not None:
                wait_token(eng, in_guard[s])
            eng.dma_start(
                x_slot(s), x[:, s * SLOT:(s + 1) * SLOT]
            ).then_inc(sems[in_token[s][0]], 16)

        def emit_copy(eng_api, eng_wait, k):
            s = k // SEC_PER_SLAB
            eng_wait.wait_ge(pe_sem, MM_PER_SEC * (k + 1))
            if s >= OBUFS:
                wait_token(eng_wait, store_token[s - OBUFS])
            c0 = (k % SEC_PER_SLAB) * SECTOR
            (eng_api.copy if eng_api is nc.scalar else eng_api.tensor_copy)(
                o_slot(s)[:, c0:c0 + SECTOR], psum_sec(k),
            ).then_inc(sems[copy_token[k][0]], 1)

        def emit_store(eng_api, eng_wait, s):
            # all four sectors of slab s copied: wait each engine's count
            done = {}
            for k in range(s * SEC_PER_SLAB, (s + 1) * SEC_PER_SLAB):
                done[copy_token[k][0]] = copy_token[k][1]
            for nm, val in done.items():
                eng_wait.wait_ge(sems[nm], val)
            if store_guard[s] is not None:
                wait_token(eng_wait, store_guard[s])
            eng_api.dma_start(
                out[:, s * SLOT:(s + 1) * SLOT], o_slot(s)
            ).then_inc(sems[store_token[s][0]], 16)

        @block.sync
        def _(sync):
            sync.dma_start(wt[:, :], w[:, :]).then_inc(w_sem, 16)
            for s in range(NSLAB):
                if LOAD_ENG[s] == "sp":
                    emit_load(sync, s)

        @block.tensor
        def _(tensor):
            tensor.wait_ge(w_sem, 16)
            for k in range(NSEC):
                s = k // SEC_PER_SLAB
                if k % SEC_PER_SLAB == 0:
                    wait_token(tensor, in_token[s])
                if k >= PSUM_DEPTH:
                    wait_token(tensor, copy_token[k - PSUM_DEPTH])
                c0 = (k % SEC_PER_SLAB) * SECTOR
                rhs = x_slot(s)
                for g in range(MM_PER_SEC):
                    nc.tensor.matmul(
                        psum_sec(k)[:, g * MM_N:(g + 1) * MM_N],
                        wt[:, :],
                        rhs[:, c0 + g * MM_N:c0 + (g + 1) * MM_N],
                        start=True, stop=True,
                    ).then_inc(pe_sem, 1)

        # ACT and POOL interleave several roles; order items by pipeline
        # position so every wait is satisfied by earlier-program items.
        def build_program(engine_name):
            items = []  # (key, seq, kind, payload)
            seq = 0
            for s in range(NSLAB):
                if LOAD_ENG[s] == engine_name:
                    items.append((max(0, SEC_PER_SLAB * (s - LOAD_PREFETCH)),
                                  seq, "load", s)); seq += 1
            for k in range(NSEC):
                if COPY_ENG[k] == engine_name:
                    items.append((k, seq, "copy", k)); seq += 1
            for s in range(NSLAB):
                if STORE_ENG[s] == engine_name:
                    items.append((SEC_PER_SLAB * s + SEC_PER_SLAB,
                                  seq, "store", s)); seq += 1
            items.sort(key=lambda it: (it[0], it[1]))
            return items

        @block.scalar
        def _(scalar):
            for key, _seq, kind, v in build_program("act"):
                if kind == "load":
                    emit_load(scalar, v)
                elif kind == "copy":
                    emit_copy(nc.scalar, scalar, v)
                else:
                    emit_store(nc.scalar, scalar, v)

        @block.vector
        def _(vector):
            for k in range(NSEC):
                if COPY_ENG[k] == "dve":
                    emit_copy(nc.vector, vector, k)

        @block.gpsimd
        def _(gpsimd):
            for key, _seq, kind, v in build_program("pool"):
                if kind == "copy":
                    emit_copy(nc.gpsimd, gpsimd, v)
                else:
                    emit_store(nc.gpsimd, gpsimd, v)

    return nc


def _step_matrices(F, Q, H, R, P0):
    """Host-side P/K recursion (float64). Returns per-step (A_t, B_t) with
    x_t = x_{t-1} @ A_t + y_t @ B_t, plus the x0 propagators."""
    d = F.shape[0]
    I = np.eye(d)
    P = P0.astype(np.float64)
    F64, Q64, H64, R64 = (m.astype(np.float64) for m in (F, Q, H, R))
    As, Bs = [], []
    for _ in range(T):
        P = F64 @ P @ F64.T + Q64
        S = H64 @ P @ H64.T + R64
        K = P @ H64.T @ np.linalg.inv(S)
        As.append(((I - K @ H64) @ F64).T)
        Bs.append(K.T)
        P = (I - K @ H64) @ P
    return As, Bs


def _scalar_gains(As, Bs):
    """If every A_t/B_t is c*I, return (a[T], b[T]) else None."""
    a, b = np.empty(T), np.empty(T)
    I = np.eye(D)
    for t in range(T):
        ca, cb = As[t][0, 0], Bs[t][0, 0]
        if not (np.allclose(As[t], ca * I, atol=1e-9) and
                np.allclose(Bs[t], cb * I, atol=1e-9)):
            return None
        a[t], b[t] = ca, cb
    return a, b


def _weight_matrix(a, b):
    W = np.zeros((T, T))
    for t in range(T):
        acc = 1.0
        W[t, t] = b[t]
        for s in range(t - 1, -1, -1):
            acc *= a[s + 1]
            W[t, s] = b[s] * acc
    return W.astype(np.float32)


def _numpy_fallback(input_tensor, As, Bs, x0):
    """General-parameter path (never hit for the shipped inputs)."""
    y = input_tensor.astype(np.float32)
    x = np.broadcast_to(x0.astype(np.float32)[:, 0][None, :], (y.shape[0], D)).copy()
    out = np.empty_like(y)
    for t in range(T):
        x = x @ As[t].astype(np.float32) + y[:, t, :] @ Bs[t].astype(np.float32)
        out[:, t, :] = x
    return out


def _make_wblk(W):
    wblk = np.zeros((128, 128), dtype=np.float16)
    wblk[:64, :64] = W.T.astype(np.float16)
    wblk[64:, 64:] = W.T.astype(np.float16)
    return wblk


def _pack_x(x):
    """[B, T, D] f32 -> per-core [128, NCOL] f16 images.

    Row q = h*T + s holds time-step s of odd/even (h) batch rows; column
    c = P*D + j is (pair-index, feature). b = 2P + h within the core."""
    xh = x.astype(np.float16)
    xr = xh.reshape(NCORES, NPAIR, 2, T, D).transpose(0, 2, 3, 1, 4)
    xr = xr.reshape(NCORES, 128, NCOL)
    return [np.ascontiguousarray(xr[c]) for c in range(NCORES)]


def _unpack_core(raw):
    """[128, NCOL] f16 -> [BS, T, D] f32 (inverse of _pack_x row mapping)."""
    o = np.asarray(raw).reshape(2, T, NPAIR, D).transpose(2, 0, 1, 3)
    return np.ascontiguousarray(o.reshape(BS, T, D)).astype(np.float32)


def _x0_correction(out, a, x0):
    if np.any(x0 != 0.0):
        alpha = np.cumprod(a).astype(np.float32)          # [T]
        out = out + alpha[None, :, None] * x0[:, 0][None, None, :]
    return out


def prepare_in_maps_and_nc(inputs):
    """Build (in_maps, nc) for the fast path. Raises if the fast path does
    not apply (used by the sim harness; kernel() handles the fallback)."""
    F = np.asarray(inputs["transition_matrix"], dtype=np.float32)
    Q = np.asarray(inputs["transition_covariance"], dtype=np.float32)
    H = np.asarray(inputs["observation_matrix"], dtype=np.float32)
    R = np.asarray(inputs["observation_covariance"], dtype=np.float32)
    P0 = np.asarray(inputs["error_covariance"], dtype=np.float32)
    As, Bs = _step_matrices(F, Q, H, R, P0)
    a, b = _scalar_gains(As, Bs)
    wblk = _make_wblk(_weight_matrix(a, b))
    x = np.asarray(inputs["input_tensor"], dtype=np.float32)
    in_maps = [{"x": xc, "w": wblk} for xc in _pack_x(x)]
    if "nc" not in _CACHE:
        _CACHE["nc"] = build_nc()
    return in_maps, _CACHE["nc"]


def postprocess_core_out(raw, inputs):
    F = np.asarray(inputs["transition_matrix"], dtype=np.float32)
    Q = np.asarray(inputs["transition_covariance"], dtype=np.float32)
    H = np.asarray(inputs["observation_matrix"], dtype=np.float32)
    R = np.asarray(inputs["observation_covariance"], dtype=np.float32)
    P0 = np.asarray(inputs["error_covariance"], dtype=np.float32)
    x0 = np.asarray(inputs["state_estimate"], dtype=np.float32)
    As, Bs = _step_matrices(F, Q, H, R, P0)
    a, b = _scalar_gains(As, Bs)
    return _x0_correction(_unpack_core(raw), a, x0)


def kernel(input_tensor, transition_matrix, transition_covariance,
           observation_matrix, observation_covariance,
           state_estimate, error_covariance):
    input_tensor = np.asarray(input_tensor, dtype=np.float32)
    F = np.asarray(transition_matrix, dtype=np.float32)
    Q = np.asarray(transition_covariance, dtype=np.float32)
    H = np.asarray(observation_matrix, dtype=np.float32)
    R = np.asarray(observation_covariance, dtype=np.float32)
    x0 = np.asarray(state_estimate, dtype=np.float32)
    P0 = np.asarray(error_covariance, dtype=np.float32)

    As, Bs = _step_matrices(F, Q, H, R, P0)
    sg = _scalar_gains(As, Bs)
    if sg is None:
        return _numpy_fallback(input_tensor, As, Bs, x0)

    a, b = sg
    wblk = _make_wblk(_weight_matrix(a, b))

    from concourse.bass_utils import run_bass_kernel_spmd

    if "nc" not in _CACHE:
        _CACHE["nc"] = build_nc()
    nc = _CACHE["nc"]

    in_maps = [{"x": xc, "w": wblk} for xc in _pack_x(input_tensor)]
    res = run_bass_kernel_spmd(nc, in_maps, list(range(NCORES)))
    out = np.concatenate(
        [_unpack_core(res.results[c]["out"]) for c in range(NCORES)], axis=0
    )
    return _x0_correction(out, a, x0)
